# revision 27
# baseline (speedup 1.0000x reference)
"""Bidirectional DSS/Mamba block on 8 trn2 cores (Bass/Tile).

Sharding: core = (batch b = core//2, d_inner half = core%2). Each core
computes the full in-proj for its batch (x is needed in full for x_proj),
scans its 256 d_inner channels in both directions, and produces a partial
(256-channel) contribution to the output projection; the host sums the two
partials per batch. The only cross-core dependency is the global-gate
pooled vector, AllGather'd (bf16) over core pairs.

Layout: channels on partitions, sequence L on the free dim. All weight
transposes/permutations happen on the host. W_in/W_xproj columns are
permuted per core so the core's own d-half always occupies x-tiles 0..1,
keeping the program identical across cores (single SPMD NEFF).

Engine split (the DVE tensor_tensor_scan is the serial bottleneck at
~2.26 ns/element and supports no fast modes): scans are split between
DVE and GpSimd (K_GPN of the 16 states per d-tile go to GpSimd), the
elementwise muls (dbu = dt*x*B_n, h *= C_n) run on DVE in bf16 (2x mode)
or GpSimd per K_DBU/K_CREP. dA_n = exp(A[:,n]*dt) is one ACT op with a
per-partition scale; B_n/C_n partition-broadcasts are paired-row DMAs
from a DRAM bounce, spread over three queues. The readout y = sum_n
C_n*h_n accumulates in PSUM via PE identity matmuls with the D*x skip
folded in as a diag(D) matmul.

Loop order is d-tile-outer / state-inner so the pooled gate vector m
(computed by a fused scalar_tensor_tensor with accum_out) is complete
~2 ops after the last scan, minimizing the serial AllGather tail. The
gate is applied by scaling W_out rows (per-partition scalars) after a
PE-transpose of the gate row, avoiding any DRAM round trip.
"""

import os
import sys

sys.path.insert(0, "/opt/trn_rl_repo")

from contextlib import ExitStack

import ml_dtypes
import numpy as np

import concourse.bass as bass
import concourse.bacc as bacc
import concourse.tile as tile
from concourse import mybir
from concourse.bass_utils import run_bass_kernel_spmd

F32 = mybir.dt.float32
F16 = mybir.dt.float16
BF16 = mybir.dt.bfloat16
AF = mybir.ActivationFunctionType
OP = mybir.AluOpType

B, L, DM, DS, DI, R = 4, 900, 256, 16, 512, 16
DH = DI // 2          # d_inner channels per core
NDT = DH // 128       # 128-channel tiles per core (2)
FCH = [(0, 512), (512, L - 512)]  # PSUM-bank-aligned L chunks

# GpSimd elementwise is a trap: its Q7 muls run ~3.2us AND slow concurrent
# DVE scans from 2.03us to 3.5us (SBUF contention). Keep it idle.
K_DBU = os.environ.get("K_DBU", "dve")       # dbu mul engine (dve|gp|mix)
K_CREP = os.environ.get("K_CREP", "dve")     # crep mul engine (dve|gp|mix)


def _bcast_pair(bcp, n):
    """Partition-broadcast AP reading rows (2n, 2n+1) of bcp 128 times:
    dst [128, 2, 900]."""
    return bass.AP(
        tensor=bcp.tensor,
        offset=bcp.offset + 2 * n * L,
        ap=[[0, 128], [L, 2], [1, L]],
    )


def _build_module(shared_a: bool):
    nc = bacc.Bacc("TRN2", num_devices=8)

    ein = lambda n, s: nc.dram_tensor(n, s, F32, kind="ExternalInput")
    ein_bf = lambda n, s: nc.dram_tensor(n, s, BF16, kind="ExternalInput")
    hsT = ein_bf("hsT", [DM, L])
    WinxT = ein_bf("WinxT", [DM, DI])
    WinzT = ein_bf("WinzT", [DM, DH])
    WxT = ein_bf("WxT", [DI, R + 2 * DS])
    WdtT = ein_bf("WdtT", [R, DH])
    bdt = ein("bdt", [128, NDT])
    Afc = ein("Afc", [128, NDT * DS])      # A_f columns per (dtile, n)
    Abc = ein("Abc", [128, NDT * DS])      # A_b columns
    Ddf = ein_bf("Ddf", [DH, 128])
    Ddb = ein_bf("Ddb", [DH, 128])
    I128 = ein_bf("I128", [128, 128])
    G2T = ein_bf("G2T", [2 * DI, 2 * DH])
    bgate_sb = ein("bgate_sb", [128, 2 * NDT])
    WoT = ein_bf("WoT", [2 * DH, DM])
    outp = nc.dram_tensor("outp", [DM, L], F32, kind="ExternalOutput")

    # [4,128] / [8,128] row layouts: contiguous 128-element DMA rows (the
    # [128,4] partition-gather form costs 128 tiny descriptors ≈ 14us)
    u_cc_in = nc.dram_tensor("u_cc_in", [2 * NDT, 128], BF16, kind="Internal")
    u_cc_out = nc.dram_tensor("u_cc_out", [4 * NDT, 128], BF16, kind="Internal")

    with ExitStack() as ctx:
        tc = ctx.enter_context(tile.TileContext(nc))
        wpool = ctx.enter_context(tc.tile_pool(name="weights", bufs=1))
        apool = ctx.enter_context(tc.tile_pool(name="acts", bufs=1))
        dpool = ctx.enter_context(tc.tile_pool(name="dram", bufs=1, space="DRAM"))

        def load(name, dram, p, f, dt_=None, eng=None):
            ts = []
            for i in range(0, p, 128):
                pp = min(128, p - i)
                t = wpool.tile([pp, f], dt_ or dram.dtype, tag=f"{name}{i}", name=f"{name}{i}")
                (eng or nc.sync).dma_start(out=t, in_=dram[i : i + pp, :])
                ts.append(t)
            return ts

        # order matters: the in-proj inputs lead and split across the sync
        # and scalar queues so hs and winx stream in parallel; the gate/out
        # weights (needed much later) go on the gpsimd queue
        hs = load("hs", hsT, DM, L)
        winx = load("winx", WinxT, DM, DI, eng=nc.scalar)
        winz = load("winz", WinzT, DM, DH)
        wx = load("wx", WxT, DI, R + 2 * DS, eng=nc.scalar)
        wdt = load("wdt", WdtT, R, DH, eng=nc.gpsimd)
        bdt_s = load("bdt", bdt, 128, NDT, eng=nc.gpsimd)[0]
        af_s = load("afc", Afc, 128, NDT * DS, eng=nc.gpsimd)[0]
        ab_s = load("abc", Abc, 128, NDT * DS, eng=nc.gpsimd)[0]
        ddf = load("ddf", Ddf, DH, 128, eng=nc.sync)
        ddb = load("ddb", Ddb, DH, 128, eng=nc.sync)
        ident = load("ident", I128, 128, 128, eng=nc.sync)[0]
        wo = load("wo", WoT, 2 * DH, DM, eng=nc.gpsimd)
        g2 = load("g2", G2T, 2 * DI, 2 * DH, eng=nc.gpsimd)
        bgate_t = load("bgate", bgate_sb, 128, 2 * NDT, eng=nc.gpsimd)[0]
        ones11 = wpool.tile([1, 1], BF16, tag="ones11", name="ones11")
        nc.vector.memset(ones11, 1.0)

        # ---- in-proj: x (full DI, silu'd; own half = tiles 0..1) + z half ----
        xT = [apool.tile([128, L], BF16, tag=f"xT{i}", name=f"xT{i}") for i in range(4)]
        zg = [apool.tile([128, L], BF16, tag=f"zg{i}", name=f"zg{i}") for i in range(NDT)]
        dtT = [apool.tile([128, L], BF16, tag=f"dtT{i}", name=f"dtT{i}") for i in range(NDT)]
        w2 = [apool.tile([128, L], BF16, tag=f"w2{i}", name=f"w2{i}") for i in range(NDT)]
        bc = [apool.tile([128, 2, L], BF16, tag=f"bc{n}", name=f"bc{n}") for n in range(DS)]
        with tc.tile_pool(name="ps_early", bufs=2, space="PSUM") as ps_early:
            for pc in range(6):
                ps = ps_early.tile([128, L], F32, tag="xz", name="xz")
                for f0, fl in FCH:
                    for kc in range(2):
                        lhsT = (
                            winx[kc][:, pc * 128 : (pc + 1) * 128]
                            if pc < 4
                            else winz[kc][:, (pc - 4) * 128 : (pc - 3) * 128]
                        )
                        nc.tensor.matmul(
                            ps[:, f0 : f0 + fl],
                            lhsT,
                            hs[kc][:, f0 : f0 + fl],
                            start=(kc == 0),
                            stop=(kc == 1),
                        )
                dst = xT[pc] if pc < 4 else zg[pc - 4]
                nc.scalar.activation(dst, ps, AF.Silu)

            # ---- x_proj -> x_dbl [48, L]; bounce B/C rows (paired) to DRAM ----
            xdbl = apool.tile([R + 2 * DS, L], BF16, tag="xdbl", name="xdbl")
            ps = ps_early.tile([R + 2 * DS, L], F32, tag="aux", name="aux")
            for f0, fl in FCH:
                for kc in range(4):
                    nc.tensor.matmul(
                        ps[:, f0 : f0 + fl],
                        wx[kc],
                        xT[kc][:, f0 : f0 + fl],
                        start=(kc == 0),
                        stop=(kc == 3),
                    )
            nc.vector.tensor_copy(xdbl, ps[0 : R + 2 * DS, :])
            # bcp rows (2n, 2n+1) = (B_n, C_n)
            bcp = dpool.tile([2 * DS, L], BF16, tag="bcp", name="bcp")
            nc.sync.dma_start(
                out=bass.AP(tensor=bcp.tensor, offset=bcp.offset,
                            ap=[[2 * L, DS], [1, L]]),
                in_=xdbl[R : R + DS, :],
            )
            nc.sync.dma_start(
                out=bass.AP(tensor=bcp.tensor, offset=bcp.offset + L,
                            ap=[[2 * L, DS], [1, L]]),
                in_=xdbl[R + DS : R + 2 * DS, :],
            )
            # prefetch all B/C partition-broadcasts on sync+gpsimd only —
            # the scalar queue must stay clear for the softplus/exp chain
            for n in range(DS):
                eng = [nc.sync, nc.gpsimd][n % 2]
                eng.dma_start(out=bc[n], in_=_bcast_pair(bcp, n))

            # ---- dt = softplus(dt_r @ WdtT + bdt), table-grouped ----
            # softplus(v+b) = ln(1 + exp(v+b)) in fp32 (bf16 would cancel
            # 1+e^v for small dt); downcast only the final dt. Both dtc run
            # per ACT table so Exp/Ln each load once.
            sp = [apool.tile([128, L], F32, tag=f"sp{i}", name=f"sp{i}")
                  for i in range(NDT)]
            for dtc in range(NDT):
                ps = ps_early.tile([128, L], F32, tag="aux", name="aux")
                for f0, fl in FCH:
                    nc.tensor.matmul(
                        ps[:, f0 : f0 + fl],
                        wdt[0][:, dtc * 128 : (dtc + 1) * 128],
                        xdbl[0:R, f0 : f0 + fl],
                        start=True,
                        stop=True,
                    )
                nc.scalar.activation(
                    sp[dtc], ps, AF.Exp, bias=bdt_s[:, dtc : dtc + 1]
                )
            for dtc in range(NDT):
                # ln(e^{v+b} + 1) — the +1 rides in the Ln bias
                nc.scalar.activation(dtT[dtc], sp[dtc], AF.Ln, bias=1.0)
                # w = dt * x_own
                nc.vector.tensor_mul(w2[dtc], dtT[dtc], xT[dtc])

        # ---- scan loop: dtile-outer, state-inner ----
        m_sb = apool.tile([128, 4], F32, tag="m", name="m")  # cols f0,f1,b0,b1
        yg = [apool.tile([128, L], BF16, tag=f"yg{c}", name=f"yg{c}")
              for c in range(4)]
        with tc.tile_pool(name="ps_y", bufs=1, space="PSUM") as ps_y, \
             tc.tile_pool(name="da", bufs=4) as da_pool, \
             tc.tile_pool(name="dbu", bufs=5) as dbu_pool, \
             tc.tile_pool(name="h", bufs=6) as h_pool:
            for dtc in range(NDT):
                yp = {}
                for dr in range(2):
                    t = ps_y.tile([128, L], F32, tag=f"y{dr}{dtc}", name=f"y{dr}{dtc}")
                    yp[dr] = t
                    dd = (ddf if dr == 0 else ddb)[dtc]
                    for f0, fl in FCH:
                        nc.tensor.matmul(
                            t[:, f0 : f0 + fl],
                            dd,
                            xT[dtc][:, f0 : f0 + fl],
                            start=True,
                            stop=False,
                            skip_group_check=True,
                        )
                def mk_dbu(n):
                    # dbu = (dt*x) * B_n, issued 2 iterations ahead so the
                    # producing queue never stalls the scan chain
                    eng = {"dve": nc.vector, "gp": nc.gpsimd,
                           "mix": (nc.gpsimd if n % 2 else nc.vector)}[K_DBU]
                    t = dbu_pool.tile([128, L], BF16, tag="dbu", name="dbu")
                    eng.tensor_mul(t, w2[dtc], bc[n][:, 0, :])
                    return t

                dbu_q = [mk_dbu(0), mk_dbu(1)]
                for n in range(DS):
                    col = dtc * DS + n
                    crep = bc[n][:, 1, :]
                    # fp16 dA: enough mantissa for decay rates near 1 (bf16
                    # is not: ~1.9e-2 end-to-end error vs 9e-4 for fp16),
                    # and 2-byte operands halve the scan's SBUF traffic
                    daf = da_pool.tile([128, L], F16, tag="daf", name="daf")
                    nc.scalar.activation(
                        daf, dtT[dtc], AF.Exp, scale=af_s[:, col : col + 1]
                    )
                    if shared_a:
                        dab = daf
                    else:
                        dab = da_pool.tile([128, L], F16, tag="dab", name="dab")
                        nc.scalar.activation(
                            dab, dtT[dtc], AF.Exp, scale=ab_s[:, col : col + 1]
                        )
                    if n + 2 < DS:
                        dbu_q.append(mk_dbu(n + 2))
                    dbu = dbu_q[n]
                    hf = h_pool.tile([128, L], BF16, tag="hf", name="hf")
                    hb = h_pool.tile([128, L], BF16, tag="hb", name="hb")
                    nc.vector.tensor_tensor_scan(hf, daf, dbu, 0.0, OP.mult, OP.add)
                    nc.vector.tensor_tensor_scan(
                        hb[:, ::-1], dab[:, ::-1], dbu[:, ::-1], 0.0, OP.mult, OP.add
                    )
                    c_eng = nc.gpsimd if K_CREP == "gp" else nc.vector
                    c_eng.tensor_mul(hf, hf, crep)
                    c_eng.tensor_mul(hb, hb, crep)
                    for dr, h in ((0, hf), (1, hb)):
                        for f0, fl in FCH:
                            nc.tensor.matmul(
                                yp[dr][:, f0 : f0 + fl],
                                ident,
                                h[:, f0 : f0 + fl],
                                start=False,
                                stop=(n == DS - 1),
                                skip_group_check=True,
                            )
                # gate input: yg = y*zg, pooled sum into m column, fused
                for dr in range(2):
                    c = 2 * dr + dtc
                    nc.vector.scalar_tensor_tensor(
                        yg[c], yp[dr], 1.0, zg[dtc], OP.mult, OP.mult,
                        accum_out=m_sb[:, c : c + 1],
                    )

        with tc.tile_pool(name="ps_tail", bufs=1, space="PSUM") as ps_tail:
            # touch Sigmoid now: the table-load lands here (ACT idle, last
            # scans still running) instead of on the post-AllGather path
            sig_d = apool.tile([1, 1], F32, tag="sigd", name="sigd")
            nc.scalar.activation(sig_d, ones11, AF.Sigmoid)
            # pairwise AllGather of m (bf16), then one G2=Wgate@Wglobal matvec.
            # m is transposed on the PE so the DMA moves 4 contiguous rows.
            m_bf = apool.tile([128, 4], BF16, tag="mbf", name="mbf")
            nc.vector.tensor_copy(m_bf, m_sb)
            mT_ps = ps_tail.tile([4, 128], BF16, tag="mT", name="mT")
            nc.tensor.matmul(mT_ps, m_bf, ident, is_transpose=True)
            m4 = apool.tile([4, 128], BF16, tag="m4", name="m4")
            nc.vector.tensor_copy(m4, mT_ps)
            nc.sync.dma_start(out=u_cc_in[:, :], in_=m4)
            nc.gpsimd.collective_compute(
                "AllGather",
                OP.bypass,
                replica_groups=[[0, 1], [2, 3], [4, 5], [6, 7]],
                ins=[u_cc_in[:, :]],
                outs=[u_cc_out[:, :]],
            )
            u8 = apool.tile([8, 128], BF16, tag="u8", name="u8")
            nc.sync.dma_start(out=u8, in_=u_cc_out[:, :])
            uT_ps = ps_tail.tile([128, 8], BF16, tag="uT", name="uT")
            nc.tensor.matmul(uT_ps, u8, ident[0:8, 0:8], is_transpose=True)
            u2 = apool.tile([128, 8], BF16, tag="u2", name="u2")
            nc.vector.tensor_copy(u2, uT_ps)

            # v as a single [1,512] row: u2 columns stationary, G2 tiles stream
            vps = ps_tail.tile([1, 2 * DH], F32, tag="vps", name="vps")
            for kc in range(8):
                nc.tensor.matmul(
                    vps,
                    u2[:, kc : kc + 1],
                    g2[kc],
                    start=(kc == 0),
                    stop=(kc == 7),
                )
            vrow = apool.tile([1, 2 * DH], BF16, tag="vrow", name="vrow")
            nc.vector.tensor_copy(vrow, vps)
            # transpose v row -> [128, 4] via K=1 matmuls (no DRAM bounce)
            vT = ps_tail.tile([128, 4], F32, tag="vT", name="vT")
            for kc in range(4):
                nc.tensor.matmul(
                    vT[:, kc : kc + 1],
                    vrow[0:1, kc * 128 : (kc + 1) * 128],
                    ones11,
                    start=True,
                    stop=True,
                    skip_group_check=True,
                )
            g_sb = apool.tile([128, 4], F32, tag="g", name="g")
            nc.vector.tensor_add(g_sb, vT, bgate_t)
            nc.scalar.activation(g_sb, g_sb, AF.Sigmoid)

            # fold gate into W_out rows (per-partition scalars), then out-proj
            wo_s = [apool.tile([128, DM], BF16, tag=f"wos{kc}", name=f"wos{kc}")
                    for kc in range(4)]
            for kc in range(4):
                nc.vector.tensor_scalar_mul(
                    wo_s[kc], wo[kc], g_sb[:, kc : kc + 1]
                )
            out_sb = [apool.tile([128, L], F32, tag=f"o{i}", name=f"o{i}")
                      for i in range(2)]
            for pc in range(2):
                ops_ = ps_tail.tile([128, L], F32, tag=f"ops{pc}", name=f"ops{pc}")
                for f0, fl in FCH:
                    for kc in range(4):
                        nc.tensor.matmul(
                            ops_[:, f0 : f0 + fl],
                            wo_s[kc][:, pc * 128 : (pc + 1) * 128],
                            yg[kc][:, f0 : f0 + fl],
                            start=(kc == 0),
                            stop=(kc == 3),
                        )
                # stage on different engines so the two copies overlap
                if pc == 0:
                    nc.scalar.activation(out_sb[pc], ops_, AF.Copy)
                else:
                    nc.vector.tensor_copy(out_sb[pc], ops_)
                (nc.sync if pc == 0 else nc.scalar).dma_start(
                    out=outp[pc * 128 : (pc + 1) * 128, :], in_=out_sb[pc]
                )

    nc.finalize()
    return nc


_NC_CACHE = {}


def _get_module(shared_a: bool):
    if shared_a not in _NC_CACHE:
        _NC_CACHE[shared_a] = _build_module(shared_a)
    return _NC_CACHE[shared_a]


def kernel(**inputs):
    inp = {k: np.asarray(v, dtype=np.float32) for k, v in inputs.items()}
    hs = inp["hidden_states"]
    W_in, W_x, W_dt = inp["W_in"], inp["W_xproj"], inp["W_dt"]
    b_dt = inp["b_dt"]
    A_f = -np.exp(inp["A_log_f"])      # (512, 16)
    A_b = -np.exp(inp["A_log_b"])
    D_f, D_b = inp["D_f"], inp["D_b"]
    W_g, b_g = inp["W_global"], inp["b_global"]
    W_gate, b_gate = inp["W_gate"], inp["b_gate"]
    W_out = inp["W_out"]

    shared_a = bool(np.array_equal(A_f, A_b))
    I = np.eye(128, dtype=np.float32)
    in_maps = []
    for core in range(8):
        b, h = core // 2, core % 2
        o = h * DH                      # own-half offset in d_inner
        perm = np.r_[o : o + DH, (DH - o) % DI : (DH - o) % DI + DH]  # own first
        ownc = np.r_[o : o + DH, DI + o : DI + o + DH]  # own rows of 2*DI concat
        ccorder = np.r_[0:DH, DI : DI + DH, DH:DI, DI + DH : 2 * DI]

        def acol(A):
            # [128, NDT*DS]: col (dtc*DS + n) = A[own dtile dtc, n]
            a = A[o : o + DH].reshape(NDT, 128, DS)
            return np.ascontiguousarray(a.transpose(1, 0, 2).reshape(128, NDT * DS))

        bf = ml_dtypes.bfloat16
        bg_full = (b_gate[ownc] + W_gate[ownc] @ b_g).astype(np.float32)
        m = {
            "hsT": np.ascontiguousarray(hs[b].T).astype(bf),
            "WinxT": np.ascontiguousarray(W_in[:DI][perm].T).astype(bf),
            "WinzT": np.ascontiguousarray(W_in[DI + o : DI + o + DH].T).astype(bf),
            "WxT": np.ascontiguousarray(W_x[:, perm].T).astype(bf),
            "WdtT": np.ascontiguousarray(W_dt[o : o + DH].T).astype(bf),
            "bdt": np.ascontiguousarray(b_dt[o : o + DH].reshape(NDT, 128).T),
            "Afc": acol(A_f),
            "Abc": acol(A_b),
            "Ddf": _diag_stack(D_f[o : o + DH]).astype(bf),
            "Ddb": _diag_stack(D_b[o : o + DH]).astype(bf),
            "I128": I.astype(bf),
            "G2T": np.ascontiguousarray(
                (W_gate[ownc] @ W_g[:, ccorder] / np.float32(L)).T
            ).astype(bf),
            "bgate_sb": np.ascontiguousarray(bg_full.reshape(4, 128).T),
            "WoT": np.ascontiguousarray(W_out[:, ownc].T).astype(bf),
        }
        in_maps.append(m)

    nc = _get_module(shared_a)
    res = run_bass_kernel_spmd(nc, in_maps, core_ids=list(range(8)))
    outs = res.results
    out = np.zeros((B, L, DM), dtype=np.float32)
    for b in range(B):
        part = outs[2 * b]["outp"] + outs[2 * b + 1]["outp"]
        out[b] = part.T
    return out


def _diag_stack(d):
    out = np.zeros((DH, 128), dtype=np.float32)
    for t in range(NDT):
        out[t * 128 : (t + 1) * 128, :] = np.diag(d[t * 128 : (t + 1) * 128])
    return out
